# revision 4
# baseline (speedup 1.0000x reference)
"""NeighborSample Trainium2 kernel.

Input  x:   (8, 64, 64, 192) f32
Output:     (8*64*64, 5, 5, 192) f32 — out[b*4096 + h*64 + w, i, j, c] =
            x[b, h+i-2, w+j-2, c] (zero-padded).

Strategy: pure DMA. Data-parallel over batch (1 sample per NeuronCore).
Per core:
  - SBUF `buf` [128, 36*192]: partition r holds padded row r cols [-2, 34)
    (left w-half), partition 64+r holds cols [30, 66) (right w-half). Pad
    columns are zeroed by memset; interior loaded with two strided DMAs.
    Partitions 0-63 are served by the even SDMA engines and 64-127 by the
    odd ones, so driving the two halves from the two HWDGE rings (sync +
    scalar) keeps all 16 engines busy.
  - For each vertical shift i (5) and each w-half (2): one 3-dim SBUF->HBM
    DMA whose source is an overlapping sliding window ([C,32] step over a
    [1,960] contiguous read) and whose dest is the strided output view
    out[h, w_half, i, :, :]. Column zero padding falls out of the buffer
    layout; row zero padding handled by 4 small DMAs from a zero buffer.
Total HBM traffic per core: ~3.3 MB read + 78.6 MB write (roofline-limited
by the write side).
"""

import sys

for _p in ("/opt/trn_rl_repo",):
    if _p not in sys.path:
        sys.path.insert(0, _p)

import numpy as np

import concourse.bass as bass
import concourse.mybir as mybir
from concourse.bass_utils import run_bass_kernel_spmd

B = 8
H = W = 64
C = 192
K = 5
PAD = 2
HALF = 32            # output w positions per half
COLS = 36            # columns stored per half buffer (PAD + 32 + PAD)
ROW = COLS * C       # 6912 elems per partition
WIN = K * C          # 960: one sliding-window read / one (i, w) output chunk
OUT_W = K * K * C    # 4800
OUT_H = W * OUT_W    # 307200
XROW = W * C         # 12288
DATA = (HALF + PAD) * C  # 6528 elems of real data per half-row

N_DMA = 2 + 2 * K + 4    # loads + main stores + zero-row stores
SEM_TOTAL = 16 * N_DMA


def _emit_half(eng, out, buf, zbuf, dma_sem, half):
    """Emit the 5 main store DMAs + boundary zero stores for one w-half."""
    for i in range(K):
        # valid output rows h: source row r = h + i - PAD must be in [0, H)
        h0 = max(0, PAD - i)
        h1 = min(H, H + PAD - i)
        cnt = h1 - h0
        part0 = half * H + (h0 + i - PAD)
        eng.dma_start(
            out=bass.AP(
                out,
                h0 * OUT_H + half * HALF * OUT_W + i * WIN,
                [[OUT_H, cnt], [OUT_W, HALF], [1, WIN]],
            ),
            in_=bass.AP(buf, part0 * ROW, [[ROW, cnt], [C, HALF], [1, WIN]]),
        ).then_inc(dma_sem, 16)
    # zero rows (full w range, done once: only by half 0 at top, half 1 at bottom)
    if half == 0:
        zrows = [(i, 0, PAD - i) for i in range(PAD)]          # i=0: h 0..2, i=1: h 0..1
    else:
        zrows = [(i, H + PAD - i, i - PAD) for i in range(PAD + 1, K)]  # bottom
    for i, hz, zcnt in zrows:
        eng.dma_start(
            out=bass.AP(
                out,
                hz * OUT_H + i * WIN,
                [[OUT_H, zcnt], [OUT_W, W], [1, WIN]],
            ),
            in_=bass.AP(zbuf, 0, [[WIN, zcnt * W], [1, WIN]]),
        ).then_inc(dma_sem, 16)


def build_nc() -> bass.Bass:
    nc = bass.Bass()
    x = nc.declare_dram_parameter("x", [H, W, C], mybir.dt.float32, isOutput=False)
    out = nc.declare_dram_parameter(
        "out", [H, W, K, K, C], mybir.dt.float32, isOutput=True
    )

    with (
        nc.Block() as block,
        nc.semaphore("dve_sem") as dve_sem,
        nc.semaphore("dma_sem") as dma_sem,
        nc.sbuf_tensor("buf", [2 * H, ROW], mybir.dt.float32) as buf,
        nc.sbuf_tensor("zbuf", [2 * H, WIN], mybir.dt.float32) as zbuf,
    ):

        @block.vector
        def _(vector):
            vector.memset(bass.AP(buf, 0, [[ROW, 2 * H], [1, ROW]]), 0.0).then_inc(
                dve_sem, 1
            )
            vector.memset(bass.AP(zbuf, 0, [[WIN, 2 * H], [1, WIN]]), 0.0).then_inc(
                dve_sem, 1
            )

        @block.sync
        def _(sync):
            sync.wait_ge(dve_sem, 2)
            # interior load, left halves -> partitions 0..63 (data starts
            # after the 2 zero pad cols)
            sync.dma_start(
                out=bass.AP(buf, PAD * C, [[ROW, H], [1, DATA]]),
                in_=bass.AP(x, 0, [[XROW, H], [1, DATA]]),
            ).then_inc(dma_sem, 16)
            sync.wait_ge(dma_sem, 32)
            _emit_half(sync, out, buf, zbuf, dma_sem, half=0)
            sync.wait_ge(dma_sem, SEM_TOTAL)

        @block.scalar
        def _(scalar):
            scalar.wait_ge(dve_sem, 2)
            # interior load, right halves -> partitions 64..127 (position 0
            # is col 30; cols 64,65 stay zero from memset)
            scalar.dma_start(
                out=bass.AP(buf, H * ROW, [[ROW, H], [1, DATA]]),
                in_=bass.AP(x, (HALF - PAD) * C, [[XROW, H], [1, DATA]]),
            ).then_inc(dma_sem, 16)
            scalar.wait_ge(dma_sem, 32)
            _emit_half(scalar, out, buf, zbuf, dma_sem, half=1)
            scalar.wait_ge(dma_sem, SEM_TOTAL)

    return nc


_NC_CACHE = None


def kernel(x) -> np.ndarray:
    global _NC_CACHE
    x = np.asarray(x, dtype=np.float32)
    assert x.shape == (B, H, W, C), x.shape
    if _NC_CACHE is None:
        _NC_CACHE = build_nc()
    in_maps = [{"x": np.ascontiguousarray(x[i])} for i in range(B)]
    res = run_bass_kernel_spmd(_NC_CACHE, in_maps, list(range(B)))
    outs = [res.results[i]["out"].reshape(H * W, K, K, C) for i in range(B)]
    return np.concatenate(outs, axis=0)


# revision 8
# speedup vs baseline: 1.9534x; 1.9534x over previous
"""NeighborSample Trainium2 kernel.

Input  x:   (8, 64, 64, 192) f32
Output:     (8*64*64, 5, 5, 192) f32 — out[b*4096 + h*64 + w, i, j, c] =
            x[b, h+i-2, w+j-2, c] (zero-padded).

Strategy: pure DMA. Data-parallel over batch (1 sample per NeuronCore).
Per core:
  - SBUF `buf` [128, 36*192]: partition r holds padded row r cols [-2, 34)
    (left w-half), partition 64+r holds cols [30, 66) (right w-half). Pad
    columns are zeroed by memset; interior loaded with two strided DMAs.
    Partitions 0-63 are served by the even SDMA engines and 64-127 by the
    odd ones, so driving the two halves from the two HWDGE rings (sync +
    scalar) keeps all 16 engines busy.
  - For each vertical shift i (5) and each w-half (2): one 3-dim SBUF->HBM
    DMA whose source is an overlapping sliding window ([C,32] step over a
    [1,960] contiguous read) and whose dest is the strided output view
    out[h, w_half, i, :, :]. Column zero padding falls out of the buffer
    layout; row zero padding handled by 4 small DMAs from a zero buffer.
Total HBM traffic per core: ~3.3 MB read + 78.6 MB write (roofline-limited
by the write side).
"""

import sys

for _p in ("/opt/trn_rl_repo",):
    if _p not in sys.path:
        sys.path.insert(0, _p)

import numpy as np

import concourse.bass as bass
import concourse.mybir as mybir
from concourse.bass_utils import run_bass_kernel_spmd

B = 8
H = W = 64
C = 192
K = 5
PAD = 2
HALF = 32            # output w positions per half
COLS = 36            # columns stored per half buffer (PAD + 32 + PAD)
ROW = COLS * C       # 6912 elems per partition
WIN = K * C          # 960: one sliding-window read / one (i, w) output chunk
OUT_W = K * K * C    # 4800
OUT_H = W * OUT_W    # 307200
XROW = W * C         # 12288
DATA = (HALF + PAD) * C  # 6528 elems of real data per half-row
ZROW = 2 * WIN       # zero-buffer row: enough for a 2-row zero fill

# loads + main stores (2 h-segments except i=2) + zero-row stores
N_DMA = 2 + 2 * (2 * K - 1) + 4
SEM_TOTAL = 16 * N_DMA


def _emit_half(eng, out, buf, zbuf, dma_sem, half):
    """Emit the 5 main store DMAs + boundary zero stores for one w-half."""
    # NOTE on engine fan-out: the HWDGE splits one DMA across n SDMA engines
    # where n is the largest divisor of the OUTERMOST dim count that is
    # <= 16. SBUF APs need the partition dim outermost, so keep h outer and
    # chop the h range into segments whose counts have a divisor close to
    # 16 (62 -> 48+14, 63 -> 48+15, 64 -> 64). Without this, the 62-row
    # stores land on 2 engines and serialize (measured: 842 us).
    for i in range(K):
        # valid output rows h: source row r = h + i - PAD must be in [0, H)
        h0 = max(0, PAD - i)
        h1 = min(H, H + PAD - i)
        cnt = h1 - h0
        part0 = half * H + (h0 + i - PAD)
        segs = [cnt] if cnt % 16 == 0 else [cnt - cnt % 16, cnt % 16]
        s0 = 0
        for seg in segs:
            eng.dma_start(
                out=bass.AP(
                    out,
                    (h0 + s0) * OUT_H + half * HALF * OUT_W + i * WIN,
                    [[OUT_H, seg], [OUT_W, HALF], [1, WIN]],
                ),
                in_=bass.AP(
                    buf, (part0 + s0) * ROW, [[ROW, seg], [C, HALF], [1, WIN]]
                ),
            ).then_inc(dma_sem, 16)
            s0 += seg
    # zero rows (full w range, done once: only by half 0 at top, half 1 at bottom)
    if half == 0:
        zrows = [(i, 0, PAD - i) for i in range(PAD)]          # i=0: h 0,1; i=1: h 0
    else:
        zrows = [(i, H + PAD - i, i - PAD) for i in range(PAD + 1, K)]  # bottom
    for i, hz, zcnt in zrows:
        eng.dma_start(
            out=bass.AP(
                out,
                hz * OUT_H + i * WIN,
                [[OUT_W, W], [OUT_H, zcnt], [1, WIN]],
            ),
            in_=bass.AP(zbuf, 0, [[ZROW, W], [WIN, zcnt], [1, WIN]]),
        ).then_inc(dma_sem, 16)


def build_nc() -> bass.Bass:
    nc = bass.Bass()
    x = nc.declare_dram_parameter("x", [H, W, C], mybir.dt.float32, isOutput=False)
    out = nc.declare_dram_parameter(
        "out", [H, W, K, K, C], mybir.dt.float32, isOutput=True
    )

    with (
        nc.Block() as block,
        nc.semaphore("dve_sem") as dve_sem,
        nc.semaphore("dma_sem") as dma_sem,
        nc.sbuf_tensor("buf", [2 * H, ROW], mybir.dt.float32) as buf,
        nc.sbuf_tensor("zbuf", [W, ZROW], mybir.dt.float32) as zbuf,
    ):

        @block.vector
        def _(vector):
            vector.memset(bass.AP(buf, 0, [[ROW, 2 * H], [1, ROW]]), 0.0).then_inc(
                dve_sem, 1
            )
            vector.memset(bass.AP(zbuf, 0, [[ZROW, W], [1, ZROW]]), 0.0).then_inc(
                dve_sem, 1
            )

        @block.sync
        def _(sync):
            sync.wait_ge(dve_sem, 2)
            # interior load, left halves -> partitions 0..63 (data starts
            # after the 2 zero pad cols)
            sync.dma_start(
                out=bass.AP(buf, PAD * C, [[ROW, H], [1, DATA]]),
                in_=bass.AP(x, 0, [[XROW, H], [1, DATA]]),
            ).then_inc(dma_sem, 16)
            sync.wait_ge(dma_sem, 32)
            _emit_half(sync, out, buf, zbuf, dma_sem, half=0)
            sync.wait_ge(dma_sem, SEM_TOTAL)

        @block.scalar
        def _(scalar):
            scalar.wait_ge(dve_sem, 2)
            # interior load, right halves -> partitions 64..127 (position 0
            # is col 30; cols 64,65 stay zero from memset)
            scalar.dma_start(
                out=bass.AP(buf, H * ROW, [[ROW, H], [1, DATA]]),
                in_=bass.AP(x, (HALF - PAD) * C, [[XROW, H], [1, DATA]]),
            ).then_inc(dma_sem, 16)
            scalar.wait_ge(dma_sem, 32)
            _emit_half(scalar, out, buf, zbuf, dma_sem, half=1)
            scalar.wait_ge(dma_sem, SEM_TOTAL)

    return nc


_NC_CACHE = None


def kernel(x) -> np.ndarray:
    global _NC_CACHE
    x = np.asarray(x, dtype=np.float32)
    assert x.shape == (B, H, W, C), x.shape
    if _NC_CACHE is None:
        _NC_CACHE = build_nc()
    in_maps = [{"x": np.ascontiguousarray(x[i])} for i in range(B)]
    res = run_bass_kernel_spmd(_NC_CACHE, in_maps, list(range(B)))
    outs = [res.results[i]["out"].reshape(H * W, K, K, C) for i in range(B)]
    return np.concatenate(outs, axis=0)
